# revision 20
# baseline (speedup 1.0000x reference)
"""MixMultiHeadAttention Trainium2 kernel.

Sharding: tensor-parallel over the 8 heads. Device h computes head h for all
batches and the partial out-projection ctx_h @ out_w[64h:64h+64, :]; the host
sums the 8 partials (the "all-reduce") and adds out_b.

Padding compaction: keys 0:1536 are ~50% masked out by padding_mask; masked
keys contribute exactly 0 (scores are multiplied by the mask via V_aug, and
the denominator accumulates the mask column), so the host drops them and
packs the survivors into 896 slots (7 chunks; max observed ~795, binomial
mean 768, sigma ~20). Pad slots carry x=0 -> K=0 -> exp(0)=1 -> x mask 0 = 0.
Keys 1536:2048 stay uncompacted (the causal boundary must stay affine for
affine_select), keys 2048:2112 are the ns block. Key layout per batch:
  chunks 0..6  = compacted region A (896 slots)
  chunks 7..10 = region B (original tokens 1536:2048, causal-masked)
  chunk  11    = ns tokens (64 wide)

Math per (batch b, head h), with LC = 1472 keys, LQ = 576:
  Kt[d, k]  = sum_i x[b, k, i] wk[i, 64h+d]          (shared for k < 1408,
  Vt[d, k]    likewise                                per-token nw for ns)
  Qt[d, q]  = sum_i x[b, tokB+q, i] wq[i, 64h+d]
  S^T[k, q] = Kt[:, k] . Qt[:, q]
  P^T       = exp(S^T / 8) * causal                   (no max subtraction: |S/8| ~ 1)
  V_aug[k]  = [V[k] * m[k], m[k]]   (m = padding mask; folds pad mask + denominator)
  ctxa^T    = V_aug^T @ P^T         -> rows 0:64 unnormalized ctx^T, row 64 = denom
  out[q, :] = (ctx^T[:, q] . ow) / denom[q]

Head schedule: consts+xt(b0) stream first so b0's KV projection runs while
the 6.3MB of ns weights (nw, fp8) stream in under the 384 small ns matmuls;
b0's attention starts ~25us in (vs ~60us when ns went first). b1/b2 pieces
weave into b0's chunk loop. Outputs ship as one flat [128, 5*512] bf16 tile
per batch (contiguous partition lines; host un-permutes + normalizes).
"""

import os
import numpy as np
import ml_dtypes

import concourse.bass as bass
import concourse.mybir as mybir
from concourse.tile import TileContext
from concourse.bass_utils import run_bass_kernel_spmd
from concourse.vector_clock import ScopedClock

B, D, H, L_NS, L_S = 8, 512, 8, 64, 2048
HD = D // H            # 64
L = L_S + L_NS         # 2112
L_SO = 512
LQ = L_SO + L_NS       # 576
Q0 = L_S - L_SO        # 1536
L_A = 896              # compacted capacity for keys 0:1536 (7 chunks)
L_B = 512              # uncompacted keys 1536:2048
L_SC = L_A + L_B       # 1408 shared keys on device
L_C = L_SC + L_NS      # 1472 total keys
NKC = (L_C + 127) // 128   # 12 key chunks (11 full + final 64-wide ns chunk)
KC_B0 = L_A // 128     # 7: first region-B chunk (causal region starts here)
NQC = 5                # query chunks of 128 (4 full + one 64)
NIC = D // 128         # 4 contraction chunks
JBW = (512, 384, 512)  # proj token-blocks: A[0:512], A[512:896], B[0:512]
JBOFF = (0, 2048, 3584)        # xt col offset of each block (4 c-chunks each)
KVOFF = (0, 512, 896)          # kvt col offset of each block
XT_W = 4 * sum(JBW)            # 5632

F32 = mybir.dt.float32
BF16 = mybir.dt.bfloat16
USE_BF16 = os.environ.get("KERNEL_DTYPE", "bf16") == "bf16"
MM_DT = BF16 if USE_BF16 else F32
NP_DT = ml_dtypes.bfloat16 if USE_BF16 else np.float32
# ns weights ship as fp8e4m3 scaled by x128 (values ~N(0, 0.02^2) would land
# in the subnormal range otherwise); the 1/128 rides on xns instead.
USE_FP8_NW = USE_BF16 and os.environ.get("KERNEL_NW_FP8", "1") == "1"
NW_DT = mybir.dt.float8e4 if USE_FP8_NW else MM_DT
NW_NP_DT = ml_dtypes.float8_e4m3fn if USE_FP8_NW else NP_DT
NW_SCALE = 128.0 if USE_FP8_NW else 1.0

# ---------------------------------------------------------------------------
# Workaround: this walrus build allows at most 1 sem wait on the TileContext
# exit Drain; spread the remaining waits across preceding SP nops.
# ---------------------------------------------------------------------------
from concourse import tile as _tile_mod


def _patched_drain_and_barrier(self, tick_clock, wait_clock):
    nc = self.nc
    nops = [nc.sync.nop() for _ in range(48)]
    drain_inst = nc.sync.drain()
    wait_clock.add_sem_waits(
        drain_inst.ins, ScopedClock({None: tick_clock.global_clock})
    )
    si = drain_inst.ins.sync_info
    if si is not None and si.on_wait and len(si.on_wait) > 1:
        waits = list(si.on_wait)
        extra, keep = waits[:-1], waits[-1:]
        assert len(extra) <= len(nops), f"need {len(extra)} spare nops"
        for w, n in zip(extra, nops):
            n.ins.sync_info = mybir.SyncInfo(on_wait=[w], on_update=[])
        drain_inst.ins.sync_info = mybir.SyncInfo(
            on_wait=keep, on_update=list(si.on_update or [])
        )
    nc.all_engine_barrier()
    assert self.sems is not None
    popped = nc._tile_sem_poison_stack.pop()
    assert popped is self._sem_poison
    nc.clear_and_free_semaphores(list(self.sems.allocated().values()))
    nc.all_engine_barrier()


_orig_drain_and_barrier = _tile_mod.TileContext._drain_and_barrier
if not os.environ.get("KERNEL_FOR_SIM"):
    _tile_mod.TileContext._drain_and_barrier = _patched_drain_and_barrier


def _split_multi_waits(nc, max_waits=1):
    """This walrus build rejects instructions carrying more than one sem wait.
    Hoist extra waits onto standalone same-engine EventSemaphore (pure wait)
    instructions inserted just before the offending instruction."""
    ctr = 0
    for f in nc.m.functions:
        for bb in f.blocks:
            new = []
            for inst in bb.instructions:
                si = getattr(inst, "sync_info", None)
                waits = list(si.on_wait) if si is not None and si.on_wait else []
                if len(waits) > max_waits:
                    for w in waits[:-max_waits]:
                        ctr += 1
                        nop = mybir.InstEventSemaphore(
                            name=f"W-split-{ctr}", ins=[], outs=[]
                        )
                        nop.engine = inst.engine
                        nop.sync_info = mybir.SyncInfo(on_wait=[w], on_update=[])
                        new.append(nop)
                    inst.sync_info = mybir.SyncInfo(
                        on_wait=waits[-max_waits:],
                        on_update=list(si.on_update or []),
                    )
                new.append(inst)
            bb.instructions = new
    return ctr


def build_nc():
    nc = bass.Bass()
    # xt layout: [b, p, JBOFF[jb] + c*JBW[jb] + u] = x[b, tok(jb, u), 128c+p]
    # where tok(jb0/jb1, u) walks the compacted region-A slots and tok(jb2, u)
    # = 1536 + u. Per-batch DMA splits by jb so piece j only waits its block.
    xt_d = nc.dram_tensor("xt", [B, 128, XT_W], MM_DT, kind="ExternalInput")
    xns_d = nc.dram_tensor("xns", [128, NIC * L_NS * 8], MM_DT, kind="ExternalInput")
    wkv_d = nc.dram_tensor("wkv", [128, NIC * 128], MM_DT, kind="ExternalInput")
    wq_d = nc.dram_tensor("wq", [128, NIC * 64], MM_DT, kind="ExternalInput")
    nw_d = nc.dram_tensor("nw", [4, 128, 4 * 3072], NW_DT, kind="ExternalInput")
    ow_d = nc.dram_tensor("ow", [64, 512], mybir.dt.float32r, kind="ExternalInput")
    padf_d = nc.dram_tensor("padf", [128, B * NKC], F32, kind="ExternalInput")
    # causal mask of the parity-permuted ns-key x ns-query block
    nsmask_d = nc.dram_tensor("nsmask", [64, L_NS], MM_DT, kind="ExternalInput")
    # out_p[b, p, 512*qc + d] = out partial for query token 128*qc + p (qc=4
    # only rows 0:64 are valid); flat layout keeps DMA lines contiguous.
    out_d = nc.dram_tensor("out_p", [B, 128, NQC * 512], MM_DT, kind="ExternalOutput")
    den_d = nc.dram_tensor("den_p", [1, B * LQ], F32, kind="ExternalOutput")

    with TileContext(nc) as tc:
        with (
            tc.tile_pool(name="const", bufs=1) as cp,
            tc.tile_pool(name="nwp", bufs=4) as nwp,
            tc.tile_pool(name="nsout", bufs=1) as nso,
            tc.tile_pool(name="xp", bufs=3) as xp,
            tc.tile_pool(name="kvq", bufs=3) as kvq,
            tc.tile_pool(name="att", bufs=4) as att,
            tc.tile_pool(name="outs", bufs=2) as outs,
            tc.tile_pool(name="ps", bufs=3, space="PSUM") as ps,
            tc.tile_pool(name="psc", bufs=1, space="PSUM") as psc,
        ):
            # ---- constants / persistent tiles ----
            wkv_t = cp.tile([128, NIC * 128], MM_DT, tag="wkv")
            nc.sync.dma_start(wkv_t[:], wkv_d[:])
            wq_t = cp.tile([128, NIC * 64], MM_DT, tag="wq")
            nc.sync.dma_start(wq_t[:], wq_d[:])
            ow_t = cp.tile([64, 512], mybir.dt.float32r, tag="ow")
            nc.sync.dma_start(ow_t[:], ow_d[:])
            padf_t = cp.tile([128, B * NKC], F32, tag="padf")
            nc.sync.dma_start(padf_t[:], padf_d[:])
            nsmask_t = cp.tile([64, L_NS], MM_DT, tag="nsmask")
            nc.sync.dma_start(nsmask_t[:], nsmask_d[:])
            xns_t = cp.tile([128, NIC * L_NS * 8], MM_DT, tag="xns")
            nc.sync.dma_start(xns_t[:], xns_d[:])

            xt_ts, kvt_ts, qt_ts = {}, {}, {}

            def emit_xt_part(b, j):
                if j == 0:
                    xt_ts[b] = xp.tile([128, XT_W], MM_DT, tag="xt", name=f"xt{b}")
                nc.sync.dma_start(
                    xt_ts[b][:, JBOFF[j] : JBOFF[j] + 4 * JBW[j]],
                    xt_d[b, :, JBOFF[j] : JBOFF[j] + 4 * JBW[j]],
                )

            def emit_xt_load(b):
                for j in range(3):
                    emit_xt_part(b, j)

            # xt(b0) streams right after the consts: b0's KV projection gives
            # the PE big matmuls while nw (the ns-phase long pole) arrives.
            emit_xt_load(0)

            # identity [64, 64] replicated in both partition halves (the
            # transpose lhsT sits at base partition 64, and matmul requires
            # lhsT/rhs base partitions to match)
            id64 = cp.tile([128, 64], MM_DT, tag="id64")
            nc.gpsimd.memset(id64[0:64, :], 1.0)
            nc.gpsimd.affine_select(
                out=id64[0:64, :], in_=id64[0:64, :],
                compare_op=mybir.AluOpType.is_equal, fill=0.0,
                base=0, pattern=[[-1, 64]], channel_multiplier=1,
            )
            nc.sync.dma_start(id64[64:128, :], id64[0:64, :])

            den_all = nso.tile([65, B * LQ], F32, tag="den")

            # Every PE result flows through one rotation of 2-bank PSUM tiles.
            def ps_tile(name):
                return ps.tile([128, 1024], F32, tag="s", name=name)

            # ---- ns (per-token) projections for all batches ----
            # qkns[0:64, 8n+b] = Q_ns, qkns[64:128, 8n+b] = K_ns.
            # vns packs V_ns at col 8n+b with the valid 64 dims on rows 0:64
            # for even n and rows 64:128 for odd n (the off-parity quadrant is
            # a valid but unused matmul byproduct) — this lets each group of 4
            # tokens ship with ONE full-height copy instead of 4 quadrant
            # extracts.
            qkns = nso.tile([128, L_NS * 8], MM_DT, tag="qkns")
            vns = nso.tile([128, L_NS * 8], MM_DT, tag="vns")
            qkns_v = qkns[:].rearrange("p (n b) -> p b n", b=8)

            def emit_proj_piece(b, j):
                """j in 0..2: K|V projection of token block j; j == 3: Q
                projection plus the ns splices (kvt/qt fully ready after 3)."""
                xt_t = xt_ts[b]
                if j == 0:
                    kvt_ts[b] = kvq.tile([128, L_C], MM_DT, tag="kvt", name=f"kvt{b}")
                kvt = kvt_ts[b]
                if j < 3:
                    w = JBW[j]
                    t = ps_tile(f"kv{b}_{j}")
                    for c in range(NIC):
                        nc.tensor.matmul(
                            t[:, 0:w],
                            wkv_t[:, 128 * c : 128 * (c + 1)],
                            xt_t[:, JBOFF[j] + c * w : JBOFF[j] + (c + 1) * w],
                            start=(c == 0), stop=(c == NIC - 1),
                        )
                    nc.vector.tensor_copy(
                        kvt[:, KVOFF[j] : KVOFF[j] + w], t[:, 0:w]
                    )
                    return
                qt_ts[b] = kvq.tile([64, LQ], MM_DT, tag="qt", name=f"qt{b}")
                qt = qt_ts[b]
                t = ps_tile(f"q{b}")
                for c in range(NIC):
                    nc.tensor.matmul(
                        t[0:64, 0:512],
                        wq_t[:, 64 * c : 64 * (c + 1)],
                        xt_t[:, JBOFF[2] + c * 512 : JBOFF[2] + (c + 1) * 512],
                        start=(c == 0), stop=(c == NIC - 1),
                    )
                nc.vector.tensor_copy(qt[:, 0:512], t[0:64, 0:512])
                nc.vector.tensor_copy(qt[:, 512:LQ], qkns_v[0:64, b, :])
                # ns keys land in kvt PERMUTED: cols L_SC:L_SC+32 hold the
                # even tokens, L_SC+32:L_C the odd ones (V_ns parity lives on
                # different partition halves of vns; this order keeps every
                # splice a contiguous-destination copy). The chunk-11 causal
                # mask below compensates.
                h32 = L_SC + 32
                nc.vector.tensor_copy(
                    kvt[0:64, L_SC:h32], qkns[64:128, b : L_NS * 8 : 16]
                )
                nc.vector.tensor_copy(
                    kvt[0:64, h32:L_C], qkns[64:128, 8 + b : L_NS * 8 : 16]
                )
                nc.vector.tensor_copy(
                    kvt[64:128, L_SC:h32], vns[0:64, b : L_NS * 8 : 16]
                )
                nc.vector.tensor_copy(
                    kvt[64:128, h32:L_C], vns[64:128, 8 + b : L_NS * 8 : 16]
                )

            # ---- phase 0: b0 KV projection, then ns projections ----
            emit_proj_piece(0, 0)
            emit_proj_piece(0, 1)
            emit_proj_piece(0, 2)
            for g in range(16):
                if g % 4 == 0:
                    # one 1.5MB DMA per 4 groups (16 small loads run at poor
                    # DMA efficiency and pace the whole phase)
                    nw4_t = nwp.tile([128, 4 * 3072], NW_DT, tag="nw")
                    nc.sync.dma_start(nw4_t[:], nw_d[g // 4, :, :])
                nwo = (g % 4) * 3072
                # one 2-bank tile per group of 4 tokens: [Q|K] of token 4g+i
                # -> bank A cols 8i:8i+8 (the first MM's start=True clears the
                # bank; the rest overwrite into it); [V pair pl] -> bank B
                # cols 512+16pl:528+16pl (all 128 rows: rows 0:64 carry the
                # even token, 64:128 the odd one).
                t = ps_tile(f"ns{g}")
                for pl in range(2):
                    n0 = 4 * g + 2 * pl
                    q0 = 16 * pl
                    for c in range(NIC):
                        base = 1536 * pl + 384 * c
                        x0 = c * 512 + 8 * n0
                        rhs0 = xns_t[:, x0 : x0 + 8]
                        rhs1 = xns_t[:, x0 + 8 : x0 + 16]
                        rhs01 = xns_t[:, x0 : x0 + 16]
                        st, sp = (c == 0), (c == NIC - 1)
                        nc.tensor.matmul(
                            t[:, q0 : q0 + 8],
                            nw4_t[:, nwo + base : nwo + base + 128], rhs0,
                            start=(st and pl == 0), stop=sp,
                            skip_group_check=(pl != 0),
                        )
                        nc.tensor.matmul(
                            t[:, q0 + 8 : q0 + 16],
                            nw4_t[:, nwo + base + 128 : nwo + base + 256], rhs1,
                            start=False, stop=sp, skip_group_check=True,
                        )
                        nc.tensor.matmul(
                            t[:, 512 + q0 : 528 + q0],
                            nw4_t[:, nwo + base + 256 : nwo + base + 384], rhs01,
                            start=(st and pl == 0), stop=sp,
                            skip_group_check=True,
                        )
                nc.vector.tensor_copy(qkns[:, 32 * g : 32 * g + 32], t[:, 0:32])
                nc.vector.tensor_copy(vns[:, 32 * g : 32 * g + 32], t[:, 512:544])
            # xt(b1) queues behind nw (needed once b0's chunk loop is underway)
            emit_xt_load(1)
            emit_proj_piece(0, 3)

            # ---- main per-batch attention loop ----
            # out-projection of batch b is deferred into batch b+1's chunk
            # loop (kc = 3..7) so PE isn't stalled on the ctxt copy at the
            # batch boundary.
            pending_out = []
            for b in range(B):
                kvt = kvt_ts[b]
                qt = qt_ts[b]
                if b + 2 < B:
                    emit_xt_load(b + 2)

                # weave the NEXT batch's projection pieces into this batch's
                # chunk loop (kc 4..7). Every batch carries this PE side-work,
                # which also keeps the HAM activity monitor from re-throttling
                # the PE clock mid-run.
                weave = {}
                if b + 1 < B:
                    weave = {4 + j: [(b + 1, j)] for j in range(4)}

                # 128-col stride: PV runs with a full 128-wide stationary
                # (a 65-row matmul output measures ~230ns slower than a
                # 128-row one). Cols 65:127 are zeroed; ctx rows 65:127 are
                # never read.
                vaug = kvq.tile([128, NKC * 128], MM_DT, tag="vaug")
                # zero only the 63 pad cols per chunk: disjoint from the V
                # and mask writers, so nothing serializes behind the memset
                vaug_v = vaug[:].rearrange("p (k c) -> p k c", c=128)
                nc.gpsimd.memset(vaug_v[:, :, 65:128], 0.0)
                # mask column of every chunk in one strided copy (rows 64:128
                # of the last chunk hold garbage; never read by PV)
                nc.vector.tensor_copy(
                    vaug[:, 64 :: 128], padf_t[:, NKC * b : NKC * (b + 1)]
                )
                ctx = psc.tile([128, 1024], F32, tag="ctx")
                pt_tiles = {}

                def emit_pv(k, ctx=ctx, vaug=vaug, pt_tiles=pt_tiles):
                    w2 = 128 if k < NKC - 1 else 64
                    c02 = max(0, 128 * (k - KC_B0))
                    vchunk = vaug[0:w2, 128 * k : 128 * k + 128]
                    ptk = pt_tiles[k]
                    if c02 < 512:
                        nc.tensor.matmul(
                            ctx[:, c02:512], vchunk, ptk[0:w2, c02:512],
                            start=(k == 0), stop=(k == NKC - 2),
                            skip_group_check=True,
                        )
                    nc.tensor.matmul(
                        ctx[:, 512:LQ], vchunk, ptk[0:w2, 512:LQ],
                        start=(k == 0), stop=(k == NKC - 1),
                        skip_group_check=True,
                    )

                for kc in range(NKC):
                    if kc >= 3:
                        emit_pv(kc - 3)
                    if 3 <= kc <= 7 and pending_out:
                        pending_out[kc - 3]()
                        if kc == 7:
                            pending_out = []
                    for pb, pj in weave.get(kc, []):
                        emit_proj_piece(pb, pj)
                    w = 128 if kc < NKC - 1 else 64
                    c0 = max(0, 128 * (kc - KC_B0))
                    t = ps_tile(f"s{b}_{kc}")
                    tb = t[:].bitcast(BF16) if USE_BF16 else t[:]
                    # transposed V chunk lives in bank B beyond the ns scores;
                    # its start=True clears the bank for the ns-score matmul
                    vtr = tb[0:w, 1280:1344] if USE_BF16 else tb[0:w, 640:704]
                    nc.tensor.transpose(
                        vtr, kvt[64:128, 128 * kc : 128 * kc + w], id64[64:128, :]
                    )
                    lhsT = kvt[0:64, 128 * kc : 128 * kc + w]
                    if c0 < 512:
                        nc.tensor.matmul(
                            t[0:w, c0:512], lhsT, qt[:, c0:512],
                            start=True, stop=True,
                        )
                    nc.tensor.matmul(
                        t[0:w, 512:LQ], lhsT, qt[:, 512:LQ],
                        start=False, stop=True, skip_group_check=True,
                    )
                    mcol = padf_t[0:w, NKC * b + kc : NKC * b + kc + 1]
                    nc.vector.tensor_scalar_mul(
                        vaug[0:w, 128 * kc : 128 * kc + 64], vtr, mcol
                    )
                    pt = att.tile([128, LQ], MM_DT, tag="pt", name=f"pt{b}_{kc}")
                    pt_tiles[kc] = pt
                    nc.scalar.activation(
                        pt[0:w, c0:LQ], t[0:w, c0:LQ],
                        mybir.ActivationFunctionType.Exp, scale=0.125,
                    )
                    if KC_B0 <= kc < NKC - 1:
                        # shared-q cols: keep pt[kk, f] iff global q (= f+c0)
                        # >= k - 1536 (= kk + c0), i.e. f >= kk
                        nc.gpsimd.affine_select(
                            out=pt[0:w, c0:512], in_=pt[0:w, c0:512],
                            compare_op=mybir.AluOpType.is_ge, fill=0.0,
                            base=0, pattern=[[1, 512 - c0]], channel_multiplier=-1,
                        )
                    if kc == NKC - 1:
                        # causal mask of the (parity-permuted) ns-key x
                        # ns-query block: host-precomputed 0/1 mask
                        nc.vector.tensor_mul(
                            pt[0:64, 512:LQ], pt[0:64, 512:LQ], nsmask_t[:]
                        )
                emit_pv(NKC - 3)
                emit_pv(NKC - 2)
                emit_pv(NKC - 1)

                ctxt = outs.tile([65, LQ], mybir.dt.float32r, tag="ctxt")
                nc.vector.tensor_copy(ctxt[0:64, :], ctx[0:64, 0:LQ])
                # denominator row shipped to the host (normalization happens
                # there, after the per-head gather)
                nc.vector.tensor_copy(
                    den_all[64:65, LQ * b : LQ * (b + 1)], ctx[64:65, 0:LQ]
                )
                if b == B - 1:
                    nc.sync.dma_start(den_d[:], den_all[64:65, :])

                # out-projection (float32r: full-rate fp32 matmul) + store.
                # One flat bf16 [128, 2560] tile per batch -> one contiguous
                # DMA (b7: per-qc DMAs so the store overlaps the final MMs).
                o_sb = outs.tile([128, NQC * 512], MM_DT, tag="osb")

                def emit_out(qc, b=b, ctxt=ctxt, o_sb=o_sb):
                    wq_ = 128 if qc < 4 else 64
                    t = ps_tile(f"o{b}_{qc}")
                    nc.tensor.matmul(
                        t[0:wq_, 0:512],
                        ctxt[0:64, 128 * qc : 128 * qc + wq_],
                        ow_t[:], start=True, stop=True,
                    )
                    dst = o_sb[0:wq_, 512 * qc : 512 * (qc + 1)]
                    if qc % 2 == 0:
                        nc.vector.tensor_copy(dst, t[0:wq_, 0:512])
                    else:
                        nc.scalar.copy(dst, t[0:wq_, 0:512])
                    if b == B - 1:
                        nc.sync.dma_start(
                            out_d[b, 0:wq_, 512 * qc : 512 * (qc + 1)], dst
                        )
                    elif qc == NQC - 1:
                        nc.sync.dma_start(out_d[b, :, 0:2048], o_sb[:, 0:2048])
                        nc.sync.dma_start(
                            out_d[b, 0:64, 2048:2560], o_sb[0:64, 2048:2560]
                        )

                if b == B - 1:
                    for qc in range(NQC):
                        emit_out(qc)
                else:
                    pending_out = [
                        (lambda qc=qc: emit_out(qc)) for qc in range(NQC)
                    ]
    if not os.environ.get("KERNEL_FOR_SIM"):
        _split_multi_waits(nc)
    return nc


_NC = None


def _get_nc():
    global _NC
    if _NC is None:
        _NC = build_nc()
    return _NC


def _prep_inputs(x, padding_mask, wq_sw, wk_sw, wv_sw, wq_nw, wk_nw, wv_nw, out_w):
    """Host-side layout prep. Returns per-core input dicts, or None if the
    compaction capacity is exceeded (caller falls back to numpy)."""
    # per-batch compaction of keys 0:1536 into L_A slots
    xs = np.zeros((B, L_SC, D), np.float32)
    padp = np.zeros((B, NKC * 128), np.float32)
    for b in range(B):
        idx = np.nonzero(padding_mask[b, :Q0])[0]
        nk = len(idx)
        if nk > L_A:
            return None
        xs[b, :nk] = x[b, idx, :]
        xs[b, L_A:] = x[b, Q0:L_S, :]
        padp[b, :nk] = 1.0
        padp[b, L_A:L_SC] = padding_mask[b, Q0:L_S].astype(np.float32)
        padp[b, L_SC:L_C] = 1.0

    # xt[b, p, JBOFF[jb] + c*W + u] = xs[b, blockoff(jb)+u, 128c+p]
    xT = np.ascontiguousarray(xs.transpose(0, 2, 1))          # [B, 512, 1408]
    parts = []
    toff = 0
    for w in JBW:
        seg = xT[:, :, toff : toff + w]                       # [B, 512, w]
        seg = seg.reshape(B, NIC, 128, w).transpose(0, 2, 1, 3)
        parts.append(seg.reshape(B, 128, NIC * w))
        toff += w
    xt = np.ascontiguousarray(np.concatenate(parts, axis=2)).astype(NP_DT)

    # xns[p, 512c + 8n + b] = x[b, 2048+n, 128c+p]
    xns_f = x[:, L_S:, :].transpose(2, 1, 0) / NW_SCALE      # [512, 64, 8]
    xns = np.ascontiguousarray(
        xns_f.reshape(NIC, 128, L_NS * 8)
    ).transpose(1, 0, 2).reshape(128, NIC * L_NS * 8)
    xns = np.ascontiguousarray(xns).astype(NP_DT)

    # padf_t[p, NKC*b + kc] = padp[b, 128*kc + p]
    padt = padp.reshape(B, NKC, 128).transpose(2, 0, 1).reshape(128, B * NKC)
    padt = np.ascontiguousarray(padt).astype(np.float32)

    # permuted ns causal mask: key row r < 32 is token 2r, row 32+r' is
    # token 2r'+1; query col q is ns token q; keep iff q >= key token
    tok = np.concatenate([2 * np.arange(32), 2 * np.arange(32) + 1])
    nsmask = (np.arange(L_NS)[None, :] >= tok[:, None]).astype(NP_DT)

    in_maps = []
    for h in range(H):
        sl = slice(HD * h, HD * (h + 1))
        wkv = np.concatenate([wk_sw[:, sl], wv_sw[:, sl]], axis=1)   # [512, 128]
        wkv = wkv.reshape(NIC, 128, 128).transpose(1, 0, 2).reshape(128, NIC * 128)
        wqh = wq_sw[:, sl].reshape(NIC, 128, 64).transpose(1, 0, 2).reshape(
            128, NIC * 64
        )
        # pair layout: nw[g, p, 1536*pl + 384*c + j] where pair = 2g+pl,
        # j in [0:128] = [wq|wk] of token 2*pair, [128:256] = [wq|wk] of
        # token 2*pair+1, [256:384] = [wv of 2*pair | wv of 2*pair+1]
        qk = np.concatenate([wq_nw[:, :, sl], wk_nw[:, :, sl]], axis=2)
        qk_pairs = qk.reshape(32, 2, D, 128)                 # [pair, 0/1, i, j]
        vv_pairs = (
            wv_nw[:, :, sl].reshape(32, 2, D, HD)
            .transpose(0, 2, 1, 3).reshape(32, D, 128)
        )
        block = np.concatenate(
            [qk_pairs[:, 0], qk_pairs[:, 1], vv_pairs], axis=2
        )                                                    # [32, 512, 384]
        nwg = (
            block.reshape(16, 2, NIC, 128, 384)
            .transpose(0, 3, 1, 2, 4)
        )
        nwg = np.ascontiguousarray(nwg).reshape(16, 128, 3072) * NW_SCALE
        # 4 groups concatenated per DMA: nw[g4, p, 3072*i + c] = nwg[4*g4+i, p, c]
        nwg = np.ascontiguousarray(
            nwg.reshape(4, 4, 128, 3072).transpose(0, 2, 1, 3)
        ).reshape(4, 128, 4 * 3072)
        in_maps.append(
            dict(
                xt=xt,
                xns=xns,
                wkv=np.ascontiguousarray(wkv).astype(NP_DT),
                wq=np.ascontiguousarray(wqh).astype(NP_DT),
                nw=nwg.astype(NW_NP_DT),
                ow=np.ascontiguousarray(out_w[sl, :]).astype(np.float32),
                padf=padt,
                nsmask=nsmask,
            )
        )
    return in_maps


def _reference_numpy(x, padding_mask, L_s, L_s_out, params):
    """Exact fallback (only used if inputs deviate from the expected pattern)."""
    def mix_linear(xx, Ls, sw, sb, nw, nb):
        out_s = xx[:, :Ls] @ sw + sb
        out_ns = np.einsum("bni,nio->bno", xx[:, Ls:], nw) + nb
        return np.concatenate([out_s, out_ns], axis=1)

    p = params
    Bb, Lx, d = x.shape
    hd = d // H
    K = mix_linear(x, L_s, p["wk_sw"], p["wk_sb"], p["wk_nw"], p["wk_nb"])
    V = mix_linear(x, L_s, p["wv_sw"], p["wv_sb"], p["wv_nw"], p["wv_nb"])
    K = K.reshape(Bb, Lx, H, hd).transpose(0, 2, 1, 3)
    V = V.reshape(Bb, Lx, H, hd).transpose(0, 2, 1, 3)
    if L_s_out < L_s:
        q_input = np.concatenate([x[:, L_s - L_s_out : L_s], x[:, L_s:]], axis=1)
    else:
        q_input = x
    L_q = L_s_out + (Lx - L_s)
    L_k = Lx
    Q = mix_linear(q_input, L_s_out, p["wq_sw"], p["wq_sb"], p["wq_nw"], p["wq_nb"])
    Q = Q.reshape(Bb, L_q, H, hd).transpose(0, 2, 1, 3)
    scores = np.einsum("bhqd,bhkd->bhqk", Q, K) / np.sqrt(hd).astype(np.float32)
    i = np.arange(L_q)[:, None]
    j = np.arange(L_k)[None, :]
    causal = j <= i + (L_k - L_q)
    pad = np.concatenate(
        [padding_mask.astype(bool), np.ones((Bb, Lx - L_s), bool)], axis=1
    )
    mask = causal[None, None] & pad[:, None, None, :]
    scores = np.where(mask, scores, -1e9)
    scores = scores - scores.max(axis=-1, keepdims=True)
    w = np.exp(scores)
    w = w / w.sum(axis=-1, keepdims=True)
    out = np.einsum("bhqk,bhkd->bhqd", w, V).transpose(0, 2, 1, 3).reshape(Bb, L_q, d)
    return (out @ p["out_w"] + p["out_b"]).astype(np.float32)


def kernel(
    x, padding_mask, L_s, L_s_out,
    wq_sw, wq_sb, wq_nw, wq_nb,
    wk_sw, wk_sb, wk_nw, wk_nb,
    wv_sw, wv_sb, wv_nw, wv_nb,
    out_w, out_b,
):
    x = np.asarray(x, np.float32)
    padding_mask = np.asarray(padding_mask)
    params = dict(
        wq_sw=np.asarray(wq_sw, np.float32), wq_sb=np.asarray(wq_sb, np.float32),
        wq_nw=np.asarray(wq_nw, np.float32), wq_nb=np.asarray(wq_nb, np.float32),
        wk_sw=np.asarray(wk_sw, np.float32), wk_sb=np.asarray(wk_sb, np.float32),
        wk_nw=np.asarray(wk_nw, np.float32), wk_nb=np.asarray(wk_nb, np.float32),
        wv_sw=np.asarray(wv_sw, np.float32), wv_sb=np.asarray(wv_sb, np.float32),
        wv_nw=np.asarray(wv_nw, np.float32), wv_nb=np.asarray(wv_nb, np.float32),
        out_w=np.asarray(out_w, np.float32), out_b=np.asarray(out_b, np.float32),
    )
    biases_zero = all(
        not np.any(params[k])
        for k in ["wq_sb", "wq_nb", "wk_sb", "wk_nb", "wv_sb", "wv_nb"]
    )
    if (
        int(L_s) != L_S or int(L_s_out) != L_SO or x.shape != (B, L, D)
        or not biases_zero
    ):
        return _reference_numpy(x, padding_mask, int(L_s), int(L_s_out), params)

    in_maps = _prep_inputs(
        x, padding_mask,
        params["wq_sw"], params["wk_sw"], params["wv_sw"],
        params["wq_nw"], params["wk_nw"], params["wv_nw"],
        params["out_w"],
    )
    if in_maps is None:
        return _reference_numpy(x, padding_mask, int(L_s), int(L_s_out), params)
    nc = _get_nc()
    res = run_bass_kernel_spmd(
        nc, in_maps, core_ids=list(range(H)),
        trace=bool(os.environ.get("KERNEL_TRACE")),
        tmpdir=os.environ.get("KERNEL_TMPDIR"),
    )
    global _LAST_RESULT
    _LAST_RESULT = res
    out = np.zeros((B, LQ, D), np.float32)
    for h in range(H):
        den = res.results[h]["den_p"].reshape(B, LQ, 1)
        of = np.asarray(res.results[h]["out_p"], np.float32)
        oh = of.reshape(B, 128, NQC, 512).transpose(0, 2, 1, 3).reshape(
            B, NQC * 128, 512
        )[:, :LQ]
        out += oh / den
    out += params["out_b"][None, None, :]
    return out
